# revision 2
# baseline (speedup 1.0000x reference)
"""Soft decision-tree layer (depth 4, 16 leaves) on 8 trn2 NeuronCores.

Sharding: 2-way data parallel (token halves) x 4-way expert parallel
(4 leaves per core).  Each core computes, for its 2048 tokens t and its
4 leaves l:  partial[t,:] = sum_l path_l(t) * (x[t] @ Wl[l] + bl[l]).
Host sums the 4 expert partials per token half.

Matmuls run in float32r (full PE rate at N=512, ~1e-4 input rounding);
accumulation is fp32 in PSUM/SBUF.

Per-core decision data is pre-sliced on the host so the SPMD program is
core-independent: a [1024, 16] matrix whose sigmoid columns are
  0..5   : nodes 0,1,2 (both choices)          -> level 0/1 path products
  6..9   : nodes 3+l, choice e0 (l = 0..3)     -> level-2 factor per leaf
  10..13 : nodes 7+4*e0+l, choice e1           -> level-3 factor per leaf
  14..15 : zero padding (unused)
path_l = P4_l * dec[6+l] * dec[10+l], where P4 comes from cols 0..5.
"""

import numpy as np

B, S, H = 2, 2048, 1024
DP, EP = 2, 4            # data-parallel x expert-parallel = 8 cores
T = (B * S) // DP        # 2048 tokens per core
LPC = 16 // EP           # 4 leaves per core
NT = T // 128            # 16 token tiles per core
TG = 2                   # token groups (acc working set = 8 tiles)
TPG = NT // TG           # 8 token tiles per group
KC = H // 128            # 8 contraction chunks
ND = 16                  # decision columns (14 used + 2 pad)

_prog_cache = {}


def _build_program():
    if "nc" in _prog_cache:
        return _prog_cache["nc"]

    from contextlib import ExitStack
    import concourse.bacc as bacc
    import concourse.tile as tile
    import concourse.mybir as mybir

    f32 = mybir.dt.float32
    f32r = mybir.dt.float32r
    MULT = mybir.AluOpType.mult
    ADD = mybir.AluOpType.add
    SIG = mybir.ActivationFunctionType.Sigmoid

    nc = bacc.Bacc("TRN2", target_bir_lowering=False, debug=False, num_devices=8)

    xt_d = nc.dram_tensor("xt", [H, T], f32r, kind="ExternalInput").ap()
    wl_d = nc.dram_tensor("wl", [LPC, H, H], f32r, kind="ExternalInput").ap()
    wd_d = nc.dram_tensor("wd", [H, ND], f32r, kind="ExternalInput").ap()
    bd_d = nc.dram_tensor("bd", [1, ND], f32r, kind="ExternalInput").ap()
    bl_d = nc.dram_tensor("bl", [1, LPC * H], f32r, kind="ExternalInput").ap()
    ones_d = nc.dram_tensor("ones", [1, 128], f32r, kind="ExternalInput").ap()
    out_d = nc.dram_tensor("out", [T, H], f32, kind="ExternalOutput").ap()

    with tile.TileContext(nc) as tc, ExitStack() as ctx:
        consts = ctx.enter_context(tc.tile_pool(name="consts", bufs=1))
        xt_pool = ctx.enter_context(tc.tile_pool(name="xt", bufs=1))
        wl_pool = ctx.enter_context(tc.tile_pool(name="wl", bufs=16))
        acc_pool = ctx.enter_context(tc.tile_pool(name="acc", bufs=1))
        dec_pool = ctx.enter_context(tc.tile_pool(name="dec", bufs=2))
        ps_pool = ctx.enter_context(tc.tile_pool(name="ps", bufs=4, space="PSUM"))
        dp_pool = ctx.enter_context(tc.tile_pool(name="dp", bufs=2, space="PSUM"))

        # --- constants ---
        ones = consts.tile([1, 128], f32r, tag="ones")
        nc.sync.dma_start(ones[:], ones_d[:, :])
        wd_sb = consts.tile([128, KC * ND], f32r, tag="wd")
        nc.sync.dma_start(
            wd_sb[:].rearrange("p (k n) -> p k n", k=KC),
            wd_d.rearrange("(k p) n -> p k n", p=128),
        )
        bd_sb = consts.tile([1, ND], f32r, tag="bd")
        nc.sync.dma_start(bd_sb[:], bd_d[:, :])
        bl_sb = consts.tile([1, LPC * H], f32r, tag="bl")
        nc.sync.dma_start(bl_sb[:], bl_d[:, :])

        # bl broadcast to all 128 partitions via ones-vector matmul
        blb = consts.tile([128, LPC * H], f32, tag="blb")
        for j in range(LPC * H // 512):
            bp = ps_pool.tile([128, 512], f32, tag="ps")
            nc.tensor.matmul(bp[:], ones[:], bl_sb[:, j * 512:(j + 1) * 512],
                             start=True, stop=True)
            nc.vector.tensor_copy(blb[:, j * 512:(j + 1) * 512], bp[:])

        # --- resident transposed activations, per (k-chunk, token group) ---
        xt = {}
        for g in range(TG):
            for k in range(KC):
                t_ = xt_pool.tile([128, T // TG], f32r, tag=f"xt{k}_{g}")
                nc.sync.dma_start(
                    t_[:], xt_d[k * 128:(k + 1) * 128,
                                g * (T // TG):(g + 1) * (T // TG)])
                xt[k, g] = t_

        for g in range(TG):
            # --- decisions + sigmoid for this token group ---
            dec_sb = dec_pool.tile([128, TPG * ND], f32, tag="dec")
            for t in range(TPG):
                dps = dp_pool.tile([128, ND], f32, tag="dp")
                for k in range(KC):
                    nc.tensor.matmul(
                        dps[:], xt[k, g][:, t * 128:(t + 1) * 128],
                        wd_sb[:, k * ND:(k + 1) * ND],
                        start=(k == 0), stop=False)
                nc.tensor.matmul(dps[:], ones[:], bd_sb[:],
                                 start=False, stop=True)
                nc.scalar.activation(
                    dec_sb[:, t * ND:(t + 1) * ND], dps[:], SIG)

            # --- path products (per-leaf cumulative probabilities) ---
            dec3 = dec_sb[:].rearrange("p (t n) -> p t n", n=ND)
            dec4 = dec_sb[:].rearrange("p (t n c) -> p t n c", n=ND // 2, c=2)
            p2 = dec_pool.tile([128, TPG * 2], f32, tag="p2")
            p23 = p2[:].rearrange("p (t n) -> p t n", n=2)
            nc.vector.tensor_copy(p23, dec3[:, :, 0:2])
            p4 = dec_pool.tile([128, TPG * 4], f32, tag="p4")
            p43 = p4[:].rearrange("p (t n) -> p t n", n=4)
            nc.vector.tensor_tensor(p43[:, :, 0:2], p23, dec4[:, :, 1:3, 0],
                                    op=MULT)
            nc.vector.tensor_tensor(p43[:, :, 2:4], p23, dec4[:, :, 1:3, 1],
                                    op=MULT)
            tmp = dec_pool.tile([128, TPG * 4], f32, tag="tmp")
            tmp3 = tmp[:].rearrange("p (t n) -> p t n", n=4)
            nc.vector.tensor_tensor(tmp3, p43, dec3[:, :, 6:10], op=MULT)
            path = dec_pool.tile([128, TPG * 4], f32, tag="path")
            path3 = path[:].rearrange("p (t n) -> p t n", n=4)
            nc.vector.tensor_tensor(path3, tmp3, dec3[:, :, 10:14], op=MULT)

            # --- main: 4 leaves x 8 token tiles x 8 k-chunks x 2 n-halves ---
            accs = []
            for t in range(TPG):
                a = acc_pool.tile([128, H], f32, tag=f"acc{t}")
                accs.append(a)
            for l in range(LPC):
                wls = []
                for k in range(KC):
                    w = wl_pool.tile([128, H], f32r, tag="wl")
                    nc.sync.dma_start(w[:], wl_d[l, k * 128:(k + 1) * 128, :])
                    wls.append(w)
                for t in range(TPG):
                    pcol = path[:, t * LPC + l:t * LPC + l + 1]
                    if l == 0:
                        # init acc with the path-weighted leaf biases
                        nc.vector.tensor_scalar(
                            accs[t][:], blb[:, 0:H],
                            path[:, t * LPC:t * LPC + 1], None, op0=MULT)
                        for j in range(1, LPC):
                            nc.vector.scalar_tensor_tensor(
                                accs[t][:], blb[:, j * H:(j + 1) * H],
                                path[:, t * LPC + j:t * LPC + j + 1],
                                accs[t][:], op0=MULT, op1=ADD)
                    psl = ps_pool.tile([128, 512], f32, tag="ps")
                    psr = ps_pool.tile([128, 512], f32, tag="ps")
                    for k in range(KC):
                        lhsT = xt[k, g][:, t * 128:(t + 1) * 128]
                        nc.tensor.matmul(psl[:], lhsT, wls[k][:, 0:512],
                                         start=(k == 0), stop=(k == KC - 1))
                        nc.tensor.matmul(psr[:], lhsT, wls[k][:, 512:1024],
                                         start=(k == 0), stop=(k == KC - 1))
                    nc.vector.scalar_tensor_tensor(
                        accs[t][:, 0:512], psl[:], pcol, accs[t][:, 0:512],
                        op0=MULT, op1=ADD)
                    nc.vector.scalar_tensor_tensor(
                        accs[t][:, 512:1024], psr[:], pcol, accs[t][:, 512:1024],
                        op0=MULT, op1=ADD)
            for t in range(TPG):
                r0 = (g * TPG + t) * 128
                nc.sync.dma_start(out_d[r0:r0 + 128, :], accs[t][:])

    nc.compile()
    _prog_cache["nc"] = nc
    return nc


def _core_inputs(x, Wd, bd, Wl, bl):
    """Build the 8 per-core input dicts (host-side sharding)."""
    x2 = np.ascontiguousarray(x, dtype=np.float32).reshape(B * S, H)
    Wd = np.asarray(Wd, dtype=np.float32)
    bd = np.asarray(bd, dtype=np.float32)
    Wl = np.ascontiguousarray(Wl, dtype=np.float32)
    bl = np.asarray(bl, dtype=np.float32)

    xts = [np.ascontiguousarray(x2[d * T:(d + 1) * T].T) for d in range(DP)]

    in_maps = []
    for c in range(8):
        d, e = c // EP, c % EP
        e1, e0 = e // 2, e % 2
        wd_c = np.zeros((H, ND), dtype=np.float32)
        bd_c = np.zeros((1, ND), dtype=np.float32)
        for n in range(3):                      # nodes 0,1,2 both choices
            wd_c[:, 2 * n:2 * n + 2] = Wd[n]
            bd_c[0, 2 * n:2 * n + 2] = bd[n]
        for l in range(4):
            wd_c[:, 6 + l] = Wd[3 + l, :, e0]   # level-2 factor
            bd_c[0, 6 + l] = bd[3 + l, e0]
            n3 = 7 + 4 * e0 + l                 # level-3 factor
            wd_c[:, 10 + l] = Wd[n3, :, e1]
            bd_c[0, 10 + l] = bd[n3, e1]
        in_maps.append({
            "xt": xts[d],
            "wl": np.ascontiguousarray(Wl[LPC * e:LPC * (e + 1)]),
            "wd": wd_c,
            "bd": bd_c,
            "bl": np.ascontiguousarray(
                bl[LPC * e:LPC * (e + 1)].reshape(1, LPC * H)),
            "ones": np.ones((1, 128), dtype=np.float32),
        })
    return in_maps


def kernel(x, Wd, bd, Wl, bl, _want_results=False):
    from concourse import bass_utils

    nc = _build_program()
    in_maps = _core_inputs(x, Wd, bd, Wl, bl)
    res = bass_utils.run_bass_kernel_spmd(nc, in_maps, list(range(8)))

    out = np.empty((DP, T, H), dtype=np.float32)
    for d in range(DP):
        s = np.zeros((T, H), dtype=np.float64)
        for e in range(EP):
            s += res.results[d * EP + e]["out"]
        out[d] = s.astype(np.float32)
    out = out.reshape(B, S, H)
    if _want_results:
        return out, res
    return out
